# revision 53
# baseline (speedup 1.0000x reference)
"""Trainium2 Bass kernel for nn_BatchedTeacherPolicy.

2048 independent per-teacher MLPs (obs-norm -> 48->512->256->128->12,
ELU between layers, tanh at the end). Pure data parallel: 256 teachers
per NeuronCore across 8 cores, 2 groups of 128 teachers per core.

v3 design (vs. v2 all-bf16): the kernel is HBM-bound (109% of the
358 GB/s per-core share in the v2 trace), so the only lever is bytes.
- W1 (69% of weight traffic) is stored as fp8 E3M4 (float8e3) with a
  global scale s1 (sigma -> 2); the 1/s1 dequant is folded into the L1
  epilogue's existing bias-add (scalar_tensor_tensor mult+add), and the
  PE consumes fp8 stationary weights directly against fp16 moving
  activations (fp8 also halves LDWEIGHTS time via 4/cycle FWL).
- Everything else drops bf16 -> fp16 (same bytes, 8x less rounding
  noise): W0/W2/W3, all activations, biases, and the packed
  [obs, -mean, 1/std] normalization input. This buys the error budget
  that W1-fp8 spends (measured: fp16-everything 6.1e-4, +W1-fp8
  1.42e-2 vs the 2e-2 gate; W2-fp8 as well would bust it at 2.0e-2).
- L0 stays on DVE in teacher-row space (teacher-on-partition): one
  broadcast multiply + segmented reduce per 128-output chunk. L1/L2/L3
  run on the TensorEngine in transposed activation space as
  LDWEIGHTS+1-column-MATMUL pairs per teacher, accumulating into
  [O, 128 teachers] PSUM tiles.
- ELU(y)+1 = exp(min(y,0)) + max(y,0) is computed instead of ELU; the
  -1 is folded into the next layer's bias on the host
  (b' = b - W_quantized @ 1), saving one DVE pass per layer.
- DMA is split across both HWDGE rings: SP carries the big col-space
  weight stream, ACT carries row-phase inputs + biases + outputs.
"""

from contextlib import ExitStack

import numpy as np
import ml_dtypes

import concourse.bass as bass
import concourse.bacc as bacc
import concourse.tile as tile
from concourse import mybir
from concourse.bass_utils import run_bass_kernel_spmd

N, OBS = 2048, 48
DIMS = [(512, 48), (256, 512), (128, 256), (12, 128)]  # (out, in) per layer
N_CORES = 8
NPC = N // N_CORES  # teachers per core
P = 128             # partitions = teachers per group
G = NPC // P        # groups per core

O0 = DIMS[0][0]          # 512
O1, I1 = DIMS[1]         # 256, 512
O2, I2 = DIMS[2]         # 128, 256
O3, I3 = DIMS[3]         # 12, 128
CI1 = I1 // P            # 4 contraction chunks for L1
OC1 = O1 // P            # 2 output chunks for L1
CI2 = I2 // P            # 2 contraction chunks for L2
TS1 = 64                 # teachers per W1 DMA tile (16KB/partition in fp8)
TS2 = 64                 # teachers per W2 DMA tile (16KB/partition in fp16)
OCH0 = 256               # L0 output chunk (DVE FMA path)
NCH0 = O0 // OCH0        # 2 chunks

# W1 fp8 quantization scale: sigma(W1) = 1/sqrt(512) -> 2.0. Absmax of
# W1*s1 is ~12, inside E3M4's +-15.5 normal range; host clips to +-15.
S1 = 2.0 * np.sqrt(512.0)
INV_S1 = float(1.0 / S1)

F32 = mybir.dt.float32
F16 = mybir.dt.float16
FP8 = mybir.dt.float8e3
AF = mybir.ActivationFunctionType
ALU = mybir.AluOpType
NPF16 = np.float16
NPFP8 = ml_dtypes.float8_e3m4

_cached = {}


def _build_bass():
    nc = bacc.Bacc(trn_type="TRN2", target_bir_lowering=False)

    # host-packed [obs, -mean, 1/std] along dim 1: one DMA, and the
    # negate/reciprocal disappear from the DVE critical path
    nrm_d = nc.dram_tensor("nrm", [NPC, 3, OBS], F16, kind="ExternalInput")
    # W0 host-pretransposed i-major + o-chunked: [t, ch, i, o_local]
    w0_d = nc.dram_tensor("W0", [NPC, NCH0, OBS, OCH0], F16, kind="ExternalInput")
    b0_d = nc.dram_tensor("b0", [NPC, O0], F16, kind="ExternalInput")
    # host-pretransposed weights: [g, ci, i_local(part), teacher, o]
    w1_d = nc.dram_tensor("W1T", [G, CI1, P, P, O1], FP8, kind="ExternalInput")
    b1_d = nc.dram_tensor("b1T", [G, OC1, P, P], F16, kind="ExternalInput")
    w2_d = nc.dram_tensor("W2T", [G, CI2, P, P, O2], F16, kind="ExternalInput")
    b2_d = nc.dram_tensor("b2T", [G, P, P], F16, kind="ExternalInput")
    w3_d = nc.dram_tensor("W3T", [G, P, P, O3], F16, kind="ExternalInput")
    b3_d = nc.dram_tensor("b3T", [G, O3, P], F16, kind="ExternalInput")
    # col-space output [o, t]: 12 fat descriptors per group instead of
    # 128 48-byte ones; the host transposes at gather time
    out_d = nc.dram_tensor("out", [O3, NPC], F32, kind="ExternalOutput")

    from concourse.masks import make_identity

    with ExitStack() as ctx:
        tc = ctx.enter_context(tile.TileContext(nc))
        w0pool = ctx.enter_context(tc.tile_pool(name="w0pool", bufs=3))
        wcpool = ctx.enter_context(tc.tile_pool(name="wcpool", bufs=5))
        w3pool = ctx.enter_context(tc.tile_pool(name="w3pool", bufs=2))
        xpool = ctx.enter_context(tc.tile_pool(name="xpool", bufs=2))
        spool = ctx.enter_context(tc.tile_pool(name="spool", bufs=3))
        bpool = ctx.enter_context(tc.tile_pool(name="bpool", bufs=2))
        ppool = ctx.enter_context(tc.tile_pool(name="ppool", bufs=1, space="PSUM"))
        ipool = ctx.enter_context(tc.tile_pool(name="ipool", bufs=1))

        ident_h = ipool.tile([P, P], F16)
        make_identity(nc, ident_h)

        def emit_row_dmas(g):
            """Issue this group's row-phase DMAs on the ACT ring. Emitted
            for ALL groups before any ACT compute op so a compute wait
            (e.g. the ELU Exp gated on L0) can never head-of-line block a
            later group's input stream."""
            n0 = g * P

            nrm_t = spool.tile([P, 3, OBS], F16, tag="nrm", bufs=2, name=f"nrm_{g}")
            nc.scalar.dma_start(out=nrm_t, in_=nrm_d[n0 : n0 + P, :, :])
            b0t = bpool.tile([P, O0], F16, tag="b0", name=f"b0_{g}")
            nc.scalar.dma_start(out=b0t, in_=b0_d[n0 : n0 + P, :])
            # col-space biases (small, -1 already folded in on host)
            b1t = bpool.tile([P, OC1, P], F16, tag="b1", name=f"b1_{g}")
            for oc in range(OC1):
                nc.scalar.dma_start(out=b1t[:, oc, :], in_=b1_d[g, oc])
            b2t = bpool.tile([P, P], F16, tag="b2", name=f"b2_{g}")
            nc.scalar.dma_start(out=b2t, in_=b2_d[g])
            b3t = bpool.tile([O3, P], F16, tag="b3", name=f"b3_{g}")
            nc.scalar.dma_start(out=b3t, in_=b3_d[g])
            w0ts = []
            for ch in range(NCH0):
                wt = w0pool.tile([P, OBS, OCH0], F16, tag="w0", name=f"w0_{g}_{ch}")
                nc.scalar.dma_start(out=wt, in_=w0_d[n0 : n0 + P, ch])
                w0ts.append(wt)
            return nrm_t, b0t, b1t, b2t, b3t, w0ts

        def emit_row_compute(g, dmas, inline_prep=None):
            """Row-space phase: norm + L0 on DVE -> x1' = ELU(y0)+1 (fp16).
            With inline_prep (an x1T tile), chunk 0's PE transposes are
            emitted inline (copies on the idle Scalar engine) so L1's first
            ci-pass can start ~20us before chunk 1 exists."""
            nrm_t, b0t, b1t, b2t, b3t, w0ts = dmas

            # ---- obs normalization: x0 = clip((obs-mean)/std, -5, 5) ----
            x0 = spool.tile([P, OBS], F32, tag="x0", name=f"x0_{g}")
            nc.vector.tensor_add(x0, nrm_t[:, 0, :], nrm_t[:, 1, :])
            nc.vector.tensor_mul(x0, x0, nrm_t[:, 2, :])
            x0h = xpool.tile([P, OBS], F32, tag="x0h", name=f"x0h_{g}")
            nc.vector.tensor_scalar(
                out=x0h, in0=x0, scalar1=-5.0, scalar2=5.0,
                op0=ALU.max, op1=ALU.min,
            )
            # f16 copy of x0 for the FMA chains: an all-16-bit operand set
            # keeps the DVE in its 2x mode (the fp32 x0h is only for the
            # seed tensor_scalar_mul, which requires an fp32 scalar)
            x0f = xpool.tile([P, OBS], F16, tag="x0f", name=f"x0f_{g}")
            nc.vector.tensor_copy(x0f, x0h)

            # ---- L0 as i-major FMA chains on DVE:
            # y0[t, o] = b0[t, o] + sum_i x0[t, i] * W0[t, i, o],
            # one scalar_tensor_tensor per (i, chunk, half) with the x0
            # column as the per-partition scalar. All operands fp16 (2x DVE
            # mode); 4 independent chains (2 o-chunks x 2 i-halves)
            # interleaved to hide op-to-op dependency latency.
            HI = OBS // 2
            y0 = xpool.tile([P, O0], F16, tag="y0", name=f"y0_{g}")
            y0b = xpool.tile([P, O0], F16, tag="y0b", name=f"y0b_{g}")
            x1h = xpool.tile([P, O0], F16, tag="x1h", name=f"x1h_{g}")
            # chunk-OUTER: o-chunk 0 (-> x1T ci 0,1) finishes ~20us before
            # chunk 1, so the col phase's first ci-pass can start early.
            for ch in range(NCH0):
                c0 = ch * OCH0
                for i in range(HI):
                    for acc, ioff in ((y0, 0), (y0b, HI)):
                        ysl = acc[:, c0 : c0 + OCH0]
                        if i == 0:
                            in1 = b0t[:, c0 : c0 + OCH0] if ioff == 0 else None
                        else:
                            in1 = ysl
                        if in1 is None:
                            nc.vector.tensor_scalar_mul(
                                ysl, w0ts[ch][:, i + ioff, :],
                                x0h[:, i + ioff : i + ioff + 1],
                            )
                        else:
                            nc.vector.scalar_tensor_tensor(
                                out=ysl,
                                in0=w0ts[ch][:, i + ioff, :],
                                scalar=x0f[:, i + ioff : i + ioff + 1],
                                in1=in1,
                                op0=ALU.mult,
                                op1=ALU.add,
                            )
                # combine the two i-half chains for this chunk, then
                # x1' = ELU(y0)+1 = exp(min(y0,0)) + max(y0,0), in fp16
                ysl, ybsl = y0[:, c0 : c0 + OCH0], y0b[:, c0 : c0 + OCH0]
                nc.vector.tensor_add(ysl, ysl, ybsl)
                e0 = spool.tile([P, OCH0], F32, tag="e0", name=f"e0_{g}_{ch}")
                nc.vector.tensor_scalar_min(e0, ysl, 0.0)
                nc.scalar.activation(e0, e0, AF.Exp)
                nc.vector.scalar_tensor_tensor(
                    out=x1h[:, c0 : c0 + OCH0], in0=ysl, scalar=0.0, in1=e0,
                    op0=ALU.max, op1=ALU.add,
                )
                if inline_prep is not None and ch == 0:
                    emit_prep_chunk(g, x1h, 0, inline_prep)

            # absorb the bias-DMA waits now so the ELU epilogue ops in the
            # col phase carry only their PSUM wait
            b1a = bpool.tile([P, OC1, P], F32, tag="b1a", name=f"b1a_{g}")
            nc.vector.tensor_scalar_mul(b1a, b1t, 1.0)
            b2a = bpool.tile([P, P], F32, tag="b2a", name=f"b2a_{g}")
            nc.vector.tensor_scalar_mul(b2a, b2t, 1.0)
            b3a = bpool.tile([O3, P], F32, tag="b3a", name=f"b3a_{g}")
            nc.vector.tensor_scalar_mul(b3a, b3t, 1.0)
            return x1h, b1a, b2a, b3a

        def elu_chunk(yps, babs, out_ap, g, li, oc, scale=None, yps_b=None):
            """out = exp(min(y,0)) + max(y,0), y = scale*(psum[+psum_b]) + bias.
            The fp8 dequant scale rides the existing bias-add op."""
            t0 = spool.tile([P, P], F32, tag="t0", name=f"t0_{g}_{li}_{oc}")
            if scale is None:
                nc.vector.tensor_add(t0, yps, babs)
            else:
                nc.vector.scalar_tensor_tensor(
                    out=t0, in0=yps, scalar=scale, in1=babs,
                    op0=ALU.mult, op1=ALU.add,
                )
            if yps_b is not None:
                # second partial folded separately: the DVE can read only
                # one PSUM operand per instruction
                nc.vector.scalar_tensor_tensor(
                    out=t0, in0=yps_b, scalar=scale or 1.0, in1=t0,
                    op0=ALU.mult, op1=ALU.add,
                )
            e = spool.tile([P, P], F32, tag="el", name=f"el_{g}_{li}_{oc}")
            nc.vector.tensor_scalar_min(e, t0, 0.0)
            nc.scalar.activation(e, e, AF.Exp)
            nc.vector.scalar_tensor_tensor(
                out=out_ap, in0=t0, scalar=0.0, in1=e, op0=ALU.max, op1=ALU.add,
            )

        def emit_prep_chunk(g, x1h, ch, x1T):
            """Transpose o-chunk ch of x1' into x1T columns (2 ci's); the
            SBUF copies ride the idle Scalar engine so they never block the
            DVE queue behind a PSUM wait."""
            for ci in (2 * ch, 2 * ch + 1):
                pst = ppool.tile([P, P], F16, tag="pst", bufs=2,
                                 name=f"pst_{g}_{ci}")
                nc.tensor.transpose(pst, x1h[:, ci * P : (ci + 1) * P], ident_h)
                nc.scalar.copy(x1T[:, ci, :], pst)

        def emit_x1prep(g, x1h):
            """x1' [t, 512] -> x1T [ci][i, t] (fp16) via PE transposes."""
            x1T = xpool.tile([P, CI1, P], F16, tag="x1T", name=f"x1T_{g}")
            for ch in range(NCH0):
                emit_prep_chunk(g, x1h, ch, x1T)
            return x1T

        def emit_l1_pass(g, x1T, cis, tag):
            """One ci-pass of L1 over all teachers into its OWN psum tiles
            (groups open and close within the pass); the epilogue sums the
            two passes' partials on the DVE."""
            yps1 = [
                ppool.tile([P, P], F32, tag=f"yps1{tag}_{oc}",
                           name=f"yps1{tag}_{g}_{oc}")
                for oc in range(OC1)
            ]
            for tb in range(0, P, TS1):
                wts = {}
                for ci in cis:
                    wt = wcpool.tile(
                        [P, TS1, O1], FP8, tag="wc", name=f"w1_{g}_{tb}_{ci}"
                    )
                    nc.gpsimd.dma_start(out=wt, in_=w1_d[g, ci, :, tb : tb + TS1, :])
                    wts[ci] = wt
                for tl in range(TS1):
                    t = tb + tl
                    for oc in range(OC1):
                        for ci in cis:
                            nc.tensor.matmul(
                                yps1[oc][:, t : t + 1],
                                lhsT=wts[ci][:, tl, oc * P : (oc + 1) * P],
                                rhs=x1T[:, ci, t : t + 1],
                                start=(ci == cis[0]),
                                stop=(ci == cis[-1]),
                            )
            return yps1

        def emit_col(g, x1T, b1a, b2a, b3a, yps1, yps1b):
            """Column-space phase after L1's matmuls: L1 epilogue (summing
            the two ci-pass partials), then L2/L3 per-teacher pairs."""
            n0 = g * P
            x2T = xpool.tile([P, CI2, P], F16, tag="x2T", name=f"x2T_{g}")
            for oc in range(OC1):
                elu_chunk(yps1[oc], b1a[:, oc, :], x2T[:, oc, :], g, 1, oc,
                          scale=INV_S1, yps_b=yps1b[oc])

            # W3T is tiny and lives in its own pool: issue its DMA
            # ahead of the W2T stream so it never lands in the tail.
            w3t = w3pool.tile([P, P, O3], F16, tag="w3", name=f"w3_{g}")
            nc.gpsimd.dma_start(out=w3t, in_=w3_d[g])
            # ---- L2 ----
            yps2 = ppool.tile([P, P], F32, tag="yps2", name=f"yps2_{g}")
            for tb in range(0, P, TS2):
                wts = []
                for ci in range(CI2):
                    wt = wcpool.tile(
                        [P, TS2, O2], F16, tag="wc", name=f"w2_{g}_{tb}_{ci}"
                    )
                    nc.gpsimd.dma_start(out=wt, in_=w2_d[g, ci, :, tb : tb + TS2, :])
                    wts.append(wt)
                for tl in range(TS2):
                    t = tb + tl
                    for ci in range(CI2):
                        nc.tensor.matmul(
                            yps2[:, t : t + 1],
                            lhsT=wts[ci][:, tl, :],
                            rhs=x2T[:, ci, t : t + 1],
                            start=(ci == 0),
                            stop=(ci == CI2 - 1),
                        )
            x3T = xpool.tile([P, P], F16, tag="x3T", name=f"x3T_{g}")
            elu_chunk(yps2, b2a, x3T, g, 2, 0)

            # ---- L3 ----
            yps3 = ppool.tile([O3, P], F32, tag="yps3", name=f"yps3_{g}")
            for t in range(P):
                nc.tensor.matmul(
                    yps3[:, t : t + 1],
                    lhsT=w3t[:, t, :],
                    rhs=x3T[:, t : t + 1],
                    start=True,
                    stop=True,
                )
            y3 = spool.tile([O3, P], F32, tag="y3", name=f"y3_{g}")
            nc.vector.tensor_add(y3, yps3, b3a)
            nc.scalar.activation(y3, y3, AF.Tanh)
            nc.scalar.dma_start(out=out_d[:, n0 : n0 + P], in_=y3)

        # Emission order (G=2): row0, prep0, row1, col0, prep1, col1.
        # - prep(g)'s DVE copies right after row(g) so they don't queue
        #   behind the next group's L0 on the Vector sequencer.
        # - prep(g+1)'s PE transposes AFTER col(g)'s matmuls so they don't
        #   head-of-line block the PE stream waiting on x1h(g+1).
        dmas = [emit_row_dmas(g) for g in range(G)]
        # Gate the col-weight stream on group 0's W0 having fully landed:
        # occupy every wc pool buffer with a 1-element write that reads the
        # last W0 g0 tile. The real W1 dma_starts reuse these buffers, so
        # the buffer-WAW dependency (which the scheduler cannot reorder
        # around) keeps the W1/W2 descriptor glut from crowding W0 g0 out
        # of the DMA queues during the head. (v6 trace: ungated, W0 g0 got
        # ~38% of bandwidth and L0 could not start until 43us.)
        w0last = dmas[0][5][NCH0 - 1]
        for k in range(5):
            d = wcpool.tile([P, TS1 // 2, O1], F16, tag="wc", name=f"gate_{k}")
            nc.gpsimd.tensor_copy(d[:1, 0, :1], w0last[:1, OBS - 1, :1])
        # Emission: g0's L0 chunk 0 + its x1T transposes are inline, so
        # L1 g0's first ci-pass (0,1) starts as soon as chunk 0's FMAs and
        # the first W1 tiles land (~45us) instead of waiting for all of L0
        # (~80us). Chunk 1's transposes are emitted AFTER pass A's matmuls
        # so their x1h wait can't head-of-line block the PE queue.
        x1T0 = xpool.tile([P, CI1, P], F16, tag="x1T", name="x1T_0")
        x1h0, b1a0, b2a0, b3a0 = emit_row_compute(0, dmas[0], inline_prep=x1T0)
        x1h1, b1a1, b2a1, b3a1 = emit_row_compute(1, dmas[1])
        yps1a0 = emit_l1_pass(0, x1T0, (0, 1), "a")
        emit_prep_chunk(0, x1h0, 1, x1T0)
        yps1b0 = emit_l1_pass(0, x1T0, (2, 3), "b")
        emit_col(0, x1T0, b1a0, b2a0, b3a0, yps1a0, yps1b0)
        x1T1 = emit_x1prep(1, x1h1)
        yps1a1 = emit_l1_pass(1, x1T1, (0, 1), "a")
        yps1b1 = emit_l1_pass(1, x1T1, (2, 3), "b")
        emit_col(1, x1T1, b1a1, b2a1, b3a1, yps1a1, yps1b1)

    nc.compile()
    return nc


def _get_nc():
    if "nc" not in _cached:
        _cached["nc"] = _build_bass()
    return _cached["nc"]


def _pack_core_inputs(full, c):
    """Shard + lay out one core's inputs (fp8 W1, fp16 rest)."""
    sl = slice(c * NPC, (c + 1) * NPC)
    f32 = np.float32
    # W0 -> i-major o-chunked [NPC, NCH0, 48, OCH0]
    w0 = np.ascontiguousarray(
        np.asarray(full["W0"])[sl].astype(NPF16)
        .transpose(0, 2, 1).reshape(NPC, OBS, NCH0, OCH0).transpose(0, 2, 1, 3)
    )
    w1f = np.asarray(full["W1"])[sl].astype(f32)            # [NPC, 256, 512]
    w1q = np.clip(w1f * f32(S1), -15.0, 15.0).astype(NPFP8)
    w1deq = w1q.astype(f32) * f32(INV_S1)
    w2 = np.asarray(full["W2"])[sl].astype(NPF16)           # [NPC, 128, 256]
    w3 = np.asarray(full["W3"])[sl].astype(NPF16)           # [NPC, 12, 128]
    # fold the ELU "-1" into the next layer's bias: b' = b - W_quant @ 1
    b1p = np.asarray(full["b1"])[sl] - w1deq.sum(-1)
    b2p = np.asarray(full["b2"])[sl] - w2.astype(f32).sum(-1)
    b3p = np.asarray(full["b3"])[sl] - w3.astype(f32).sum(-1)
    # W1T[g, ci, i, t, oc*128+o] = W1[g*128+t, oc*128+o, ci*128+i]
    w1t = np.ascontiguousarray(
        w1q.reshape(G, P, OC1, P, CI1, P).transpose(0, 4, 5, 1, 2, 3)
        .reshape(G, CI1, P, P, O1)
    )
    b1t = np.ascontiguousarray(
        b1p.reshape(G, P, OC1, P).transpose(0, 2, 3, 1).astype(NPF16)
    )
    w2t = np.ascontiguousarray(
        w2.reshape(G, P, P, CI2, P).transpose(0, 3, 4, 1, 2)
    )
    b2t = np.ascontiguousarray(b2p.reshape(G, P, P).transpose(0, 2, 1).astype(NPF16))
    w3t = np.ascontiguousarray(w3.reshape(G, P, O3, P).transpose(0, 3, 1, 2))
    b3t = np.ascontiguousarray(b3p.reshape(G, P, O3).transpose(0, 2, 1).astype(NPF16))
    nrm = np.stack(
        [
            np.asarray(full["obs"])[sl],
            -np.asarray(full["mean"])[sl],
            1.0 / np.asarray(full["std"])[sl],
        ],
        axis=1,
    ).astype(NPF16)
    return {
        "nrm": np.ascontiguousarray(nrm),
        "W0": w0,
        "b0": np.ascontiguousarray(np.asarray(full["b0"])[sl].astype(NPF16)),
        "W1T": w1t, "b1T": b1t,
        "W2T": w2t, "b2T": b2t,
        "W3T": w3t, "b3T": b3t,
    }


def kernel(obs, mean, std, W0, b0, W1, b1, W2, b2, W3, b3, _trace=False):
    nc = _get_nc()
    full = {
        "obs": obs, "mean": mean, "std": std,
        "W0": W0, "b0": b0, "W1": W1, "b1": b1,
        "W2": W2, "b2": b2, "W3": W3, "b3": b3,
    }
    in_maps = [_pack_core_inputs(full, c) for c in range(N_CORES)]
    res = run_bass_kernel_spmd(
        nc, in_maps, core_ids=list(range(N_CORES)), trace=_trace
    )
    _cached["last_results"] = res
    out = np.concatenate(
        [np.ascontiguousarray(res.results[c]["out"].T) for c in range(N_CORES)],
        axis=0,
    )
    return out


# revision 55
# speedup vs baseline: 1.0622x; 1.0622x over previous
"""Trainium2 Bass kernel for nn_BatchedTeacherPolicy.

2048 independent per-teacher MLPs (obs-norm -> 48->512->256->128->12,
ELU between layers, tanh at the end). Pure data parallel: 256 teachers
per NeuronCore across 8 cores, 2 groups of 128 teachers per core.

v3 design (vs. v2 all-bf16): the kernel is HBM-bound (109% of the
358 GB/s per-core share in the v2 trace), so the only lever is bytes.
- W1 (69% of weight traffic) is stored as fp8 E3M4 (float8e3) with a
  global scale s1 (sigma -> 2); the 1/s1 dequant is folded into the L1
  epilogue's existing bias-add (scalar_tensor_tensor mult+add), and the
  PE consumes fp8 stationary weights directly against fp16 moving
  activations (fp8 also halves LDWEIGHTS time via 4/cycle FWL).
- Everything else drops bf16 -> fp16 (same bytes, 8x less rounding
  noise): W0/W2/W3, all activations, biases, and the packed
  [obs, -mean, 1/std] normalization input. This buys the error budget
  that W1-fp8 spends (measured: fp16-everything 6.1e-4, +W1-fp8
  1.42e-2 vs the 2e-2 gate; W2-fp8 as well would bust it at 2.0e-2).
- L0 stays on DVE in teacher-row space (teacher-on-partition): one
  broadcast multiply + segmented reduce per 128-output chunk. L1/L2/L3
  run on the TensorEngine in transposed activation space as
  LDWEIGHTS+1-column-MATMUL pairs per teacher, accumulating into
  [O, 128 teachers] PSUM tiles.
- ELU(y)+1 = exp(min(y,0)) + max(y,0) is computed instead of ELU; the
  -1 is folded into the next layer's bias on the host
  (b' = b - W_quantized @ 1), saving one DVE pass per layer.
- DMA is split across both HWDGE rings: SP carries the big col-space
  weight stream, ACT carries row-phase inputs + biases + outputs.
"""

from contextlib import ExitStack

import numpy as np
import ml_dtypes

import concourse.bass as bass
import concourse.bacc as bacc
import concourse.tile as tile
from concourse import mybir
from concourse.bass_utils import run_bass_kernel_spmd

N, OBS = 2048, 48
DIMS = [(512, 48), (256, 512), (128, 256), (12, 128)]  # (out, in) per layer
N_CORES = 8
NPC = N // N_CORES  # teachers per core
P = 128             # partitions = teachers per group
G = NPC // P        # groups per core

O0 = DIMS[0][0]          # 512
O1, I1 = DIMS[1]         # 256, 512
O2, I2 = DIMS[2]         # 128, 256
O3, I3 = DIMS[3]         # 12, 128
CI1 = I1 // P            # 4 contraction chunks for L1
OC1 = O1 // P            # 2 output chunks for L1
CI2 = I2 // P            # 2 contraction chunks for L2
TS1 = 32                 # teachers per W1 DMA tile (8KB/partition in fp8)
TS2 = 32                 # teachers per W2 DMA tile (8KB/partition in fp16)
OCH0 = 256               # L0 output chunk (DVE FMA path)
NCH0 = O0 // OCH0        # 2 chunks

# W1 fp8 quantization scale: sigma(W1) = 1/sqrt(512) -> 2.0. Absmax of
# W1*s1 is ~12, inside E3M4's +-15.5 normal range; host clips to +-15.
S1 = 2.0 * np.sqrt(512.0)
INV_S1 = float(1.0 / S1)
# W2 ci-chunk 0 (i 0..127) also rides fp8: its dequant rides the x2T
# columns (pre-scaled by 1/S2) so both ci partials share one PSUM group.
S2 = 2.0 * np.sqrt(256.0)
INV_S2 = float(1.0 / S2)

F32 = mybir.dt.float32
F16 = mybir.dt.float16
FP8 = mybir.dt.float8e3
AF = mybir.ActivationFunctionType
ALU = mybir.AluOpType
NPF16 = np.float16
NPFP8 = ml_dtypes.float8_e3m4

_cached = {}


def _build_bass():
    nc = bacc.Bacc(trn_type="TRN2", target_bir_lowering=False)

    # host-packed [obs, -mean, 1/std] along dim 1: one DMA, and the
    # negate/reciprocal disappear from the DVE critical path
    nrm_d = nc.dram_tensor("nrm", [NPC, 3, OBS], F16, kind="ExternalInput")
    # W0 host-pretransposed i-major + o-chunked: [t, ch, i, o_local]
    w0_d = nc.dram_tensor("W0", [NPC, NCH0, OBS, OCH0], F16, kind="ExternalInput")
    b0_d = nc.dram_tensor("b0", [NPC, O0], F16, kind="ExternalInput")
    # host-pretransposed weights: [g, ci, i_local(part), teacher, o]
    w1_d = nc.dram_tensor("W1T", [G, CI1, P, P, O1], FP8, kind="ExternalInput")
    b1_d = nc.dram_tensor("b1T", [G, OC1, P, P], F16, kind="ExternalInput")
    w2q_d = nc.dram_tensor("W2Q", [G, P, P, O2], FP8, kind="ExternalInput")
    w2f_d = nc.dram_tensor("W2F", [G, P, P, O2], F16, kind="ExternalInput")
    b2_d = nc.dram_tensor("b2T", [G, P, P], F16, kind="ExternalInput")
    w3_d = nc.dram_tensor("W3T", [G, P, P, O3], F16, kind="ExternalInput")
    b3_d = nc.dram_tensor("b3T", [G, O3, P], F16, kind="ExternalInput")
    # col-space output [o, t]: 12 fat descriptors per group instead of
    # 128 48-byte ones; the host transposes at gather time
    out_d = nc.dram_tensor("out", [O3, NPC], F32, kind="ExternalOutput")

    from concourse.masks import make_identity

    with ExitStack() as ctx:
        tc = ctx.enter_context(tile.TileContext(nc))
        w0pool = ctx.enter_context(tc.tile_pool(name="w0pool", bufs=3))
        wcpool = ctx.enter_context(tc.tile_pool(name="wcpool", bufs=11))
        w3pool = ctx.enter_context(tc.tile_pool(name="w3pool", bufs=2))
        xpool = ctx.enter_context(tc.tile_pool(name="xpool", bufs=2))
        spool = ctx.enter_context(tc.tile_pool(name="spool", bufs=3))
        bpool = ctx.enter_context(tc.tile_pool(name="bpool", bufs=2))
        ppool = ctx.enter_context(tc.tile_pool(name="ppool", bufs=1, space="PSUM"))
        ipool = ctx.enter_context(tc.tile_pool(name="ipool", bufs=1))

        ident_h = ipool.tile([P, P], F16)
        make_identity(nc, ident_h)

        def emit_row_dmas(g):
            """Issue this group's row-phase DMAs on the ACT ring. Emitted
            for ALL groups before any ACT compute op so a compute wait
            (e.g. the ELU Exp gated on L0) can never head-of-line block a
            later group's input stream."""
            n0 = g * P

            nrm_t = spool.tile([P, 3, OBS], F16, tag="nrm", bufs=2, name=f"nrm_{g}")
            nc.scalar.dma_start(out=nrm_t, in_=nrm_d[n0 : n0 + P, :, :])
            b0t = bpool.tile([P, O0], F16, tag="b0", name=f"b0_{g}")
            nc.scalar.dma_start(out=b0t, in_=b0_d[n0 : n0 + P, :])
            # col-space biases (small, -1 already folded in on host)
            b1t = bpool.tile([P, OC1, P], F16, tag="b1", name=f"b1_{g}")
            for oc in range(OC1):
                nc.scalar.dma_start(out=b1t[:, oc, :], in_=b1_d[g, oc])
            b2t = bpool.tile([P, P], F16, tag="b2", name=f"b2_{g}")
            nc.scalar.dma_start(out=b2t, in_=b2_d[g])
            b3t = bpool.tile([O3, P], F16, tag="b3", name=f"b3_{g}")
            nc.scalar.dma_start(out=b3t, in_=b3_d[g])
            w0ts = []
            for ch in range(NCH0):
                wt = w0pool.tile([P, OBS, OCH0], F16, tag="w0", name=f"w0_{g}_{ch}")
                nc.scalar.dma_start(out=wt, in_=w0_d[n0 : n0 + P, ch])
                w0ts.append(wt)
            return nrm_t, b0t, b1t, b2t, b3t, w0ts

        def emit_row_compute(g, dmas, inline_prep=None):
            """Row-space phase: norm + L0 on DVE -> x1' = ELU(y0)+1 (fp16).
            With inline_prep (an x1T tile), chunk 0's PE transposes are
            emitted inline (copies on the idle Scalar engine) so L1's first
            ci-pass can start ~20us before chunk 1 exists."""
            nrm_t, b0t, b1t, b2t, b3t, w0ts = dmas

            # ---- obs normalization: x0 = clip((obs-mean)/std, -5, 5) ----
            x0 = spool.tile([P, OBS], F32, tag="x0", name=f"x0_{g}")
            nc.vector.tensor_add(x0, nrm_t[:, 0, :], nrm_t[:, 1, :])
            nc.vector.tensor_mul(x0, x0, nrm_t[:, 2, :])
            x0h = xpool.tile([P, OBS], F32, tag="x0h", name=f"x0h_{g}")
            nc.vector.tensor_scalar(
                out=x0h, in0=x0, scalar1=-5.0, scalar2=5.0,
                op0=ALU.max, op1=ALU.min,
            )
            # f16 copy of x0 for the FMA chains: an all-16-bit operand set
            # keeps the DVE in its 2x mode (the fp32 x0h is only for the
            # seed tensor_scalar_mul, which requires an fp32 scalar)
            x0f = xpool.tile([P, OBS], F16, tag="x0f", name=f"x0f_{g}")
            nc.vector.tensor_copy(x0f, x0h)

            # ---- L0 as i-major FMA chains on DVE:
            # y0[t, o] = b0[t, o] + sum_i x0[t, i] * W0[t, i, o],
            # one scalar_tensor_tensor per (i, chunk, half) with the x0
            # column as the per-partition scalar. All operands fp16 (2x DVE
            # mode); 4 independent chains (2 o-chunks x 2 i-halves)
            # interleaved to hide op-to-op dependency latency.
            HI = OBS // 2
            y0 = xpool.tile([P, O0], F16, tag="y0", name=f"y0_{g}")
            y0b = xpool.tile([P, O0], F16, tag="y0b", name=f"y0b_{g}")
            x1h = xpool.tile([P, O0], F16, tag="x1h", name=f"x1h_{g}")
            # chunk-OUTER: o-chunk 0 (-> x1T ci 0,1) finishes ~20us before
            # chunk 1, so the col phase's first ci-pass can start early.
            for ch in range(NCH0):
                c0 = ch * OCH0
                for i in range(HI):
                    for acc, ioff in ((y0, 0), (y0b, HI)):
                        ysl = acc[:, c0 : c0 + OCH0]
                        if i == 0:
                            in1 = b0t[:, c0 : c0 + OCH0] if ioff == 0 else None
                        else:
                            in1 = ysl
                        if in1 is None:
                            nc.vector.tensor_scalar_mul(
                                ysl, w0ts[ch][:, i + ioff, :],
                                x0h[:, i + ioff : i + ioff + 1],
                            )
                        else:
                            nc.vector.scalar_tensor_tensor(
                                out=ysl,
                                in0=w0ts[ch][:, i + ioff, :],
                                scalar=x0f[:, i + ioff : i + ioff + 1],
                                in1=in1,
                                op0=ALU.mult,
                                op1=ALU.add,
                            )
                # combine the two i-half chains for this chunk, then
                # x1' = ELU(y0)+1 = exp(min(y0,0)) + max(y0,0), in fp16
                ysl, ybsl = y0[:, c0 : c0 + OCH0], y0b[:, c0 : c0 + OCH0]
                nc.vector.tensor_add(ysl, ysl, ybsl)
                e0 = spool.tile([P, OCH0], F32, tag="e0", name=f"e0_{g}_{ch}")
                nc.vector.tensor_scalar_min(e0, ysl, 0.0)
                nc.scalar.activation(e0, e0, AF.Exp)
                nc.vector.scalar_tensor_tensor(
                    out=x1h[:, c0 : c0 + OCH0], in0=ysl, scalar=0.0, in1=e0,
                    op0=ALU.max, op1=ALU.add,
                )
                if inline_prep is not None and ch == 0:
                    emit_prep_chunk(g, x1h, 0, inline_prep)

            # absorb the bias-DMA waits now so the ELU epilogue ops in the
            # col phase carry only their PSUM wait
            b1a = bpool.tile([P, OC1, P], F32, tag="b1a", name=f"b1a_{g}")
            nc.vector.tensor_scalar_mul(b1a, b1t, 1.0)
            b2a = bpool.tile([P, P], F32, tag="b2a", name=f"b2a_{g}")
            nc.vector.tensor_scalar_mul(b2a, b2t, 1.0)
            b3a = bpool.tile([O3, P], F32, tag="b3a", name=f"b3a_{g}")
            nc.vector.tensor_scalar_mul(b3a, b3t, 1.0)
            return x1h, b1a, b2a, b3a

        def elu_chunk(yps, babs, out_ap, g, li, oc, scale=None, yps_b=None):
            """out = exp(min(y,0)) + max(y,0), y = scale*(psum[+psum_b]) + bias.
            The fp8 dequant scale rides the existing bias-add op."""
            t0 = spool.tile([P, P], F32, tag="t0", name=f"t0_{g}_{li}_{oc}")
            if scale is None:
                nc.vector.tensor_add(t0, yps, babs)
            else:
                nc.vector.scalar_tensor_tensor(
                    out=t0, in0=yps, scalar=scale, in1=babs,
                    op0=ALU.mult, op1=ALU.add,
                )
            if yps_b is not None:
                # second partial folded separately: the DVE can read only
                # one PSUM operand per instruction
                nc.vector.scalar_tensor_tensor(
                    out=t0, in0=yps_b, scalar=scale or 1.0, in1=t0,
                    op0=ALU.mult, op1=ALU.add,
                )
            e = spool.tile([P, P], F32, tag="el", name=f"el_{g}_{li}_{oc}")
            nc.vector.tensor_scalar_min(e, t0, 0.0)
            nc.scalar.activation(e, e, AF.Exp)
            nc.vector.scalar_tensor_tensor(
                out=out_ap, in0=t0, scalar=0.0, in1=e, op0=ALU.max, op1=ALU.add,
            )

        def emit_prep_chunk(g, x1h, ch, x1T):
            """Transpose o-chunk ch of x1' into x1T columns (2 ci's); the
            SBUF copies ride the idle Scalar engine so they never block the
            DVE queue behind a PSUM wait."""
            for ci in (2 * ch, 2 * ch + 1):
                pst = ppool.tile([P, P], F16, tag="pst", bufs=2,
                                 name=f"pst_{g}_{ci}")
                nc.tensor.transpose(pst, x1h[:, ci * P : (ci + 1) * P], ident_h)
                nc.scalar.copy(x1T[:, ci, :], pst)

        def emit_x1prep(g, x1h):
            """x1' [t, 512] -> x1T [ci][i, t] (fp16) via PE transposes."""
            x1T = xpool.tile([P, CI1, P], F16, tag="x1T", name=f"x1T_{g}")
            for ch in range(NCH0):
                emit_prep_chunk(g, x1h, ch, x1T)
            return x1T

        def emit_l1_pass(g, x1T, cis, tag):
            """One ci-pass of L1 over all teachers into its OWN psum tiles
            (groups open and close within the pass); the epilogue sums the
            two passes' partials on the DVE."""
            yps1 = [
                ppool.tile([P, P], F32, tag=f"yps1{tag}_{oc}",
                           name=f"yps1{tag}_{g}_{oc}")
                for oc in range(OC1)
            ]
            for tb in range(0, P, TS1):
                wts = {}
                for ci in cis:
                    wt = wcpool.tile(
                        [P, TS1, O1], FP8, tag="wc", name=f"w1_{g}_{tb}_{ci}"
                    )
                    nc.gpsimd.dma_start(out=wt, in_=w1_d[g, ci, :, tb : tb + TS1, :])
                    wts[ci] = wt
                for tl in range(TS1):
                    t = tb + tl
                    for oc in range(OC1):
                        for ci in cis:
                            nc.tensor.matmul(
                                yps1[oc][:, t : t + 1],
                                lhsT=wts[ci][:, tl, oc * P : (oc + 1) * P],
                                rhs=x1T[:, ci, t : t + 1],
                                start=(ci == cis[0]),
                                stop=(ci == cis[-1]),
                            )
            return yps1

        def emit_col(g, x1T, b1a, b2a, b3a, yps1, yps1b):
            """Column-space phase after L1's matmuls: L1 epilogue (summing
            the two ci-pass partials), then L2/L3 per-teacher pairs."""
            n0 = g * P
            x2T = xpool.tile([P, CI2, P], F16, tag="x2T", name=f"x2T_{g}")
            for oc in range(OC1):
                elu_chunk(yps1[oc], b1a[:, oc, :], x2T[:, oc, :], g, 1, oc,
                          scale=INV_S1, yps_b=yps1b[oc])
            nc.vector.tensor_scalar_mul(x2T[:, 0, :], x2T[:, 0, :], INV_S2)

            # W3T is tiny and lives in its own pool: issue its DMA
            # ahead of the W2T stream so it never lands in the tail.
            w3t = w3pool.tile([P, P, O3], F16, tag="w3", name=f"w3_{g}")
            nc.gpsimd.dma_start(out=w3t, in_=w3_d[g])
            # ---- L2 ----
            yps2 = ppool.tile([P, P], F32, tag="yps2", name=f"yps2_{g}")
            wqs, wfs = [], []
            for tb in range(0, P, 2 * TS2):
                wq = wcpool.tile([P, 2 * TS2, O2], FP8, tag="wc",
                                 name=f"w2q_{g}_{tb}")
                nc.gpsimd.dma_start(out=wq, in_=w2q_d[g, :, tb : tb + 2 * TS2, :])
                wqs.append(wq)
            for tb in range(0, P, TS2):
                wf = wcpool.tile([P, TS2, O2], F16, tag="wc",
                                 name=f"w2f_{g}_{tb}")
                nc.gpsimd.dma_start(out=wf, in_=w2f_d[g, :, tb : tb + TS2, :])
                wfs.append(wf)
            for t in range(P):
                nc.tensor.matmul(
                    yps2[:, t : t + 1],
                    lhsT=wqs[t // (2 * TS2)][:, t % (2 * TS2), :],
                    rhs=x2T[:, 0, t : t + 1],
                    start=True, stop=False,
                )
                nc.tensor.matmul(
                    yps2[:, t : t + 1],
                    lhsT=wfs[t // TS2][:, t % TS2, :],
                    rhs=x2T[:, 1, t : t + 1],
                    start=False, stop=True,
                )
            x3T = xpool.tile([P, P], F16, tag="x3T", name=f"x3T_{g}")
            elu_chunk(yps2, b2a, x3T, g, 2, 0)

            # ---- L3 ----
            yps3 = ppool.tile([O3, P], F32, tag="yps3", name=f"yps3_{g}")
            for t in range(P):
                nc.tensor.matmul(
                    yps3[:, t : t + 1],
                    lhsT=w3t[:, t, :],
                    rhs=x3T[:, t : t + 1],
                    start=True,
                    stop=True,
                )
            y3 = spool.tile([O3, P], F32, tag="y3", name=f"y3_{g}")
            nc.vector.tensor_add(y3, yps3, b3a)
            nc.scalar.activation(y3, y3, AF.Tanh)
            nc.scalar.dma_start(out=out_d[:, n0 : n0 + P], in_=y3)

        # Emission order (G=2): row0, prep0, row1, col0, prep1, col1.
        # - prep(g)'s DVE copies right after row(g) so they don't queue
        #   behind the next group's L0 on the Vector sequencer.
        # - prep(g+1)'s PE transposes AFTER col(g)'s matmuls so they don't
        #   head-of-line block the PE stream waiting on x1h(g+1).
        dmas = [emit_row_dmas(g) for g in range(G)]
        # Gate the col-weight stream on group 0's W0 having fully landed:
        # occupy every wc pool buffer with a 1-element write that reads the
        # last W0 g0 tile. The real W1 dma_starts reuse these buffers, so
        # the buffer-WAW dependency (which the scheduler cannot reorder
        # around) keeps the W1/W2 descriptor glut from crowding W0 g0 out
        # of the DMA queues during the head. (v6 trace: ungated, W0 g0 got
        # ~38% of bandwidth and L0 could not start until 43us.)
        w0last = dmas[0][5][NCH0 - 1]
        for k in range(11):
            d = wcpool.tile([P, TS1 // 2, O1], F16, tag="wc", name=f"gate_{k}")
            nc.gpsimd.tensor_copy(d[:1, 0, :1], w0last[:1, OBS - 1, :1])
        # Emission: g0's L0 chunk 0 + its x1T transposes are inline, so
        # L1 g0's first ci-pass (0,1) starts as soon as chunk 0's FMAs and
        # the first W1 tiles land (~45us) instead of waiting for all of L0
        # (~80us). Chunk 1's transposes are emitted AFTER pass A's matmuls
        # so their x1h wait can't head-of-line block the PE queue.
        x1T0 = xpool.tile([P, CI1, P], F16, tag="x1T", name="x1T_0")
        x1h0, b1a0, b2a0, b3a0 = emit_row_compute(0, dmas[0], inline_prep=x1T0)
        x1h1, b1a1, b2a1, b3a1 = emit_row_compute(1, dmas[1])
        yps1a0 = emit_l1_pass(0, x1T0, (0, 1), "a")
        emit_prep_chunk(0, x1h0, 1, x1T0)
        yps1b0 = emit_l1_pass(0, x1T0, (2, 3), "b")
        emit_col(0, x1T0, b1a0, b2a0, b3a0, yps1a0, yps1b0)
        x1T1 = emit_x1prep(1, x1h1)
        yps1a1 = emit_l1_pass(1, x1T1, (0, 1), "a")
        yps1b1 = emit_l1_pass(1, x1T1, (2, 3), "b")
        emit_col(1, x1T1, b1a1, b2a1, b3a1, yps1a1, yps1b1)

    nc.compile()
    return nc


def _get_nc():
    if "nc" not in _cached:
        _cached["nc"] = _build_bass()
    return _cached["nc"]


def _pack_core_inputs(full, c):
    """Shard + lay out one core's inputs (fp8 W1, fp16 rest)."""
    sl = slice(c * NPC, (c + 1) * NPC)
    f32 = np.float32
    # W0 -> i-major o-chunked [NPC, NCH0, 48, OCH0]
    w0 = np.ascontiguousarray(
        np.asarray(full["W0"])[sl].astype(NPF16)
        .transpose(0, 2, 1).reshape(NPC, OBS, NCH0, OCH0).transpose(0, 2, 1, 3)
    )
    w1f = np.asarray(full["W1"])[sl].astype(f32)            # [NPC, 256, 512]
    w1q = np.clip(w1f * f32(S1), -15.0, 15.0).astype(NPFP8)
    w1deq = w1q.astype(f32) * f32(INV_S1)
    w2f32 = np.asarray(full["W2"])[sl].astype(f32)          # [NPC, 128, 256]
    w2q = np.clip(w2f32[:, :, :P] * f32(S2), -15.0, 15.0).astype(NPFP8)
    w2deq = w2q.astype(f32) * f32(INV_S2)
    w2c1 = w2f32[:, :, P:].astype(NPF16)
    w3 = np.asarray(full["W3"])[sl].astype(NPF16)           # [NPC, 12, 128]
    # fold the ELU "-1" into the next layer's bias: b' = b - W_quant @ 1
    b1p = np.asarray(full["b1"])[sl] - w1deq.sum(-1)
    b2p = (np.asarray(full["b2"])[sl] - w2deq.sum(-1)
           - w2c1.astype(f32).sum(-1))
    b3p = np.asarray(full["b3"])[sl] - w3.astype(f32).sum(-1)
    # W1T[g, ci, i, t, oc*128+o] = W1[g*128+t, oc*128+o, ci*128+i]
    w1t = np.ascontiguousarray(
        w1q.reshape(G, P, OC1, P, CI1, P).transpose(0, 4, 5, 1, 2, 3)
        .reshape(G, CI1, P, P, O1)
    )
    b1t = np.ascontiguousarray(
        b1p.reshape(G, P, OC1, P).transpose(0, 2, 3, 1).astype(NPF16)
    )
    w2qt = np.ascontiguousarray(
        w2q.reshape(G, P, O2, P).transpose(0, 3, 1, 2)
    )
    w2ft = np.ascontiguousarray(
        w2c1.reshape(G, P, O2, P).transpose(0, 3, 1, 2)
    )
    b2t = np.ascontiguousarray(b2p.reshape(G, P, P).transpose(0, 2, 1).astype(NPF16))
    w3t = np.ascontiguousarray(w3.reshape(G, P, O3, P).transpose(0, 3, 1, 2))
    b3t = np.ascontiguousarray(b3p.reshape(G, P, O3).transpose(0, 2, 1).astype(NPF16))
    nrm = np.stack(
        [
            np.asarray(full["obs"])[sl],
            -np.asarray(full["mean"])[sl],
            1.0 / np.asarray(full["std"])[sl],
        ],
        axis=1,
    ).astype(NPF16)
    return {
        "nrm": np.ascontiguousarray(nrm),
        "W0": w0,
        "b0": np.ascontiguousarray(np.asarray(full["b0"])[sl].astype(NPF16)),
        "W1T": w1t, "b1T": b1t,
        "W2Q": w2qt, "W2F": w2ft, "b2T": b2t,
        "W3T": w3t, "b3T": b3t,
    }


def kernel(obs, mean, std, W0, b0, W1, b1, W2, b2, W3, b3, _trace=False):
    nc = _get_nc()
    full = {
        "obs": obs, "mean": mean, "std": std,
        "W0": W0, "b0": b0, "W1": W1, "b1": b1,
        "W2": W2, "b2": b2, "W3": W3, "b3": b3,
    }
    in_maps = [_pack_core_inputs(full, c) for c in range(N_CORES)]
    res = run_bass_kernel_spmd(
        nc, in_maps, core_ids=list(range(N_CORES)), trace=_trace
    )
    _cached["last_results"] = res
    out = np.concatenate(
        [np.ascontiguousarray(res.results[c]["out"].T) for c in range(N_CORES)],
        axis=0,
    )
    return out
